# revision 7
# baseline (speedup 1.0000x reference)
"""GCMC graph-conv kernel for Trainium2, 8-core SPMD.

out = ci * segment_sum((weight[node_ids] * cj)[src_idx], dst_idx)

Strategy (edge sharding by dst range):
  - core k owns dst rows [k*12500, (k+1)*12500); its edges are host-partitioned
  - device builds feat = weight * cj into DRAM scratch (streamed, sequential)
  - per dst-tile of 128 rows: messages are fetched with SWDGE dma_gather
    (int16 indices, so the feat table is split into 4 chunks of 25000 rows)
  - segment-sum is a one-hot matmul accumulated in PSUM: for each block of
    128 gathered messages, DVE builds onehot[p, d] = (iota[d] == dst_local[p])
    and TensorE does psum[d, :] += onehot.T @ msg
  - flush: ACT copies psum*ci to SBUF, HWDGE DMAs the tile out
"""
import sys, os
sys.path.insert(0, '/opt/trn_rl_repo')

import numpy as np

N_NODES = 100000
OUT_DIM = 64
N_CORES = 8
DST_PER_CORE = N_NODES // N_CORES          # 12500
N_CHUNKS = 4                                # int16 idx -> <=32767 rows per chunk
CHUNK = N_NODES // N_CHUNKS                 # 25000
PAD_SENTINEL = 999.0


def _round_up(x, m):
    return (x + m - 1) // m * m


def _host_prep(src, dst):
    """Partition edges by dst range, sort by (dst_tile, src_chunk, src), build
    per-core slot-packed gather indices and dst-local values plus the shared
    static envelope table E[tile][chunk] (max over cores, rounded to 128)."""
    n_tiles = _round_up(DST_PER_CORE, 128) // 128       # 98
    per_core = []
    counts = np.zeros((N_CORES, n_tiles, N_CHUNKS), np.int64)
    for k in range(N_CORES):
        m = (dst // DST_PER_CORE) == k
        s = src[m]
        dl = dst[m] - k * DST_PER_CORE
        t = dl // 128
        c = s // CHUNK
        order = np.lexsort((s, c, t))
        s, dl, t, c = s[order], dl[order], t[order], c[order]
        seg = t * N_CHUNKS + c
        counts[k] = np.bincount(seg, minlength=n_tiles * N_CHUNKS).reshape(
            n_tiles, N_CHUNKS)
        per_core.append((s, dl, t, c, seg))

    env = _round_up(counts.max(axis=0), 128).astype(np.int64)   # [n_tiles, N_CHUNKS]
    env_flat = env.reshape(-1)
    seg_off = np.concatenate([[0], np.cumsum(env_flat)])[:-1]
    total = int(env_flat.sum())

    idx_all, dv_all = [], []
    for k in range(N_CORES):
        s, dl, t, c, seg = per_core[k]
        seg_counts = np.bincount(seg, minlength=n_tiles * N_CHUNKS)
        within = np.arange(len(s)) - np.repeat(
            np.concatenate([[0], np.cumsum(seg_counts)])[:-1], seg_counts)
        slot = seg_off[seg] + within
        idx_flat = np.zeros(total, np.int16)
        idx_flat[slot] = (s - c * CHUNK).astype(np.int16)
        dv_flat = np.full(total, PAD_SENTINEL, np.float32)
        dv_flat[slot] = (dl - t * 128).astype(np.float32)
        # wrap idx into 16 partitions, replicate x8 (one copy per Q7 core)
        idx_w = np.tile(idx_flat.reshape(total // 16, 16).T, (8, 1)).copy()
        dv_w = dv_flat.reshape(total // 128, 128).T.copy()
        idx_all.append(idx_w)
        dv_all.append(dv_w)
    return env, seg_off, total, idx_all, dv_all


def _build_program(env, seg_off, total):
    import concourse.bass as bass
    import concourse.bacc as bacc
    import concourse.mybir as mybir
    import concourse.tile as tile

    n_tiles = env.shape[0]
    n_pad = _round_up(N_NODES, 128)                      # 100096
    n_wtiles = n_pad // 128                              # 782
    dst_pad = n_tiles * 128                              # 12544
    f32 = mybir.dt.float32

    nc = bacc.Bacc("TRN2", target_bir_lowering=False, debug=False,
                   num_devices=N_CORES)
    w_d = nc.dram_tensor("w", [n_pad, OUT_DIM], f32, kind="ExternalInput").ap()
    cj_d = nc.dram_tensor("cj", [128, n_wtiles], f32, kind="ExternalInput").ap()
    ci_d = nc.dram_tensor("ci", [128, n_tiles], f32, kind="ExternalInput").ap()
    iota_d = nc.dram_tensor("iota", [128, 128], f32, kind="ExternalInput").ap()
    idx_d = nc.dram_tensor("idx", [128, total // 16], mybir.dt.int16,
                           kind="ExternalInput").ap()
    dv_d = nc.dram_tensor("dv", [128, total // 128], f32,
                          kind="ExternalInput").ap()
    feat_d = nc.dram_tensor("feat", [n_pad, OUT_DIM], f32, kind="Internal").ap()
    out_d = nc.dram_tensor("out", [dst_pad, OUT_DIM], f32,
                           kind="ExternalOutput").ap()

    w_v = w_d.rearrange("(n p) d -> p n d", p=128)       # [128, 782, 64]
    f_v = feat_d.rearrange("(n p) d -> p n d", p=128)
    out_v = out_d.rearrange("(n p) d -> n p d", p=128)   # [98, 128, 64]

    blocks = env.sum(axis=1) // 128                      # per-tile block count
    max_blocks = int(blocks.max())

    with tile.TileContext(nc) as tc:
        with (
            tc.tile_pool(name="const", bufs=1) as constp,
            tc.tile_pool(name="wtile", bufs=3) as wpool,
            tc.tile_pool(name="msg", bufs=2) as msgp,
            tc.tile_pool(name="oh", bufs=6) as ohp,
            tc.tile_pool(name="ps", bufs=2, space="PSUM") as psp,
            tc.tile_pool(name="ot", bufs=3) as otp,
        ):
            cj_t = constp.tile([128, n_wtiles], f32)
            ci_t = constp.tile([128, n_tiles], f32)
            io_t = constp.tile([128, 128], f32)
            idx_t = constp.tile([128, total // 16], mybir.dt.int16)
            dv_t = constp.tile([128, total // 128], f32)
            nc.sync.dma_start(cj_t[:], cj_d[:])
            nc.sync.dma_start(ci_t[:], ci_d[:])
            nc.sync.dma_start(io_t[:], iota_d[:])
            nc.sync.dma_start(idx_t[:], idx_d[:])
            nc.sync.dma_start(dv_t[:], dv_d[:])

            # Phase 1: feat = w * cj, streamed in [128, G, 64] groups
            G = 8
            for i in range(0, n_wtiles, G):
                g = min(G, n_wtiles - i)
                wt = wpool.tile([128, G, OUT_DIM], f32, tag="w")
                nc.sync.dma_start(wt[:, :g, :], w_v[:, i:i + g, :])
                cj_b = cj_t[:, i:i + g].unsqueeze(2).broadcast_to(
                    [128, g, OUT_DIM])
                nc.vector.tensor_mul(wt[:, :g, :], wt[:, :g, :], cj_b)
                nc.sync.dma_start(f_v[:, i:i + g, :], wt[:, :g, :])

            # Phase 2: per dst-tile gather + one-hot matmul segment sum
            for t in range(n_tiles):
                n_blk = int(blocks[t])
                msg = msgp.tile([128, max_blocks, OUT_DIM], f32, tag="msg")
                col = 0
                for c in range(N_CHUNKS):
                    e_tc = int(env[t, c])
                    if e_tc == 0:
                        continue
                    off = int(seg_off[t * N_CHUNKS + c])
                    # single_packet packs each engine's descriptors into one
                    # packet; packets are HW-capped at 64 descs, so gathers
                    # over 1024 idxs (64*16 engines) wedge the device.
                    nc.gpsimd.dma_gather(
                        msg[:, col:col + e_tc // 128, :],
                        feat_d[c * CHUNK:c * CHUNK + CHUNK, :],
                        idx_t[:, off // 16:(off + e_tc) // 16],
                        e_tc, e_tc, OUT_DIM,
                        single_packet=(e_tc <= 1024),
                    )
                    col += e_tc // 128
                ps = psp.tile([128, OUT_DIM], f32)
                g0 = int(seg_off[t * N_CHUNKS]) // 128
                for b in range(n_blk):
                    oh = ohp.tile([128, 128], f32, tag="oh")
                    nc.vector.tensor_scalar(
                        oh[:], io_t[:], dv_t[:, g0 + b:g0 + b + 1], None,
                        mybir.AluOpType.is_equal)
                    nc.tensor.matmul(ps[:], oh[:], msg[:, b, :],
                                     start=(b == 0), stop=(b == n_blk - 1))
                ot = otp.tile([128, OUT_DIM], f32, tag="ot")
                nc.scalar.activation(ot[:], ps[:],
                                     mybir.ActivationFunctionType.Copy,
                                     scale=ci_t[:, t:t + 1])
                nc.sync.dma_start(out_v[t], ot[:])

    nc.compile()
    return nc


def prepare(node_ids, src_idx, dst_idx, cj, ci, weight):
    """Host prep + program build. Returns (nc, in_maps, postprocess)."""
    import time
    _t0 = time.time()

    node_ids = np.asarray(node_ids)
    src = np.asarray(src_idx).astype(np.int64)
    dst = np.asarray(dst_idx).astype(np.int64)
    cj = np.asarray(cj, dtype=np.float32).reshape(-1)
    ci = np.asarray(ci, dtype=np.float32).reshape(-1)
    weight = np.asarray(weight, dtype=np.float32)

    # feat rows are weight[node_ids]; with the arange fill this is identity
    if not np.array_equal(node_ids, np.arange(N_NODES, dtype=node_ids.dtype)):
        weight = weight[node_ids]

    n_pad = _round_up(N_NODES, 128)
    n_tiles = _round_up(DST_PER_CORE, 128) // 128
    w_pad = np.zeros((n_pad, OUT_DIM), np.float32)
    w_pad[:N_NODES] = weight
    cj_pad = np.zeros(n_pad, np.float32)
    cj_pad[:N_NODES] = cj
    cj_w = cj_pad.reshape(n_pad // 128, 128).T.copy()
    iota = np.tile(np.arange(128, dtype=np.float32), (128, 1))

    env, seg_off, total, idx_all, dv_all = _host_prep(src, dst)
    print(f"[kernel] host prep: {time.time()-_t0:.1f}s (total slots {total})",
          flush=True)
    _t1 = time.time()
    nc = _build_program(env, seg_off, total)
    print(f"[kernel] build+schedule+compile-to-bir: {time.time()-_t1:.1f}s",
          flush=True)
    _t2 = time.time()

    in_maps = []
    for k in range(N_CORES):
        ci_k = np.zeros(n_tiles * 128, np.float32)
        ci_k[:DST_PER_CORE] = ci[k * DST_PER_CORE:(k + 1) * DST_PER_CORE]
        ci_w = ci_k.reshape(n_tiles, 128).T.copy()
        in_maps.append({
            "w": w_pad, "cj": cj_w, "ci": ci_w, "iota": iota,
            "idx": idx_all[k], "dv": dv_all[k],
        })
    def post(results):
        return np.concatenate(
            [results[k]["out"][:DST_PER_CORE] for k in range(N_CORES)], axis=0)

    return nc, in_maps, post


def kernel(node_ids, src_idx, dst_idx, cj, ci, weight):
    import time
    from concourse.bass_utils import run_bass_kernel_spmd
    nc, in_maps, post = prepare(node_ids, src_idx, dst_idx, cj, ci, weight)
    _t2 = time.time()
    res = run_bass_kernel_spmd(nc, in_maps, core_ids=list(range(N_CORES)))
    print(f"[kernel] neff compile+exec: {time.time()-_t2:.1f}s", flush=True)
    return post(res.results)
